# revision 8
# baseline (speedup 1.0000x reference)
"""VQ codebook nearest-neighbor kernel for Trainium2 (8 NeuronCores, data-parallel).

Problem: z [2048,64,256] f32, E [1024,256] f32 ->
         out[b,u,:] = E[argmin_k ||z[b,u]-E[k]||^2]

Strategy (v6):
  - Shard z along batch across 8 cores (16384 tokens each); replicate E.
  - argmin_k ||z-e_k||^2 == argmin_k (-z.e_k + ||e_k||^2/2).  The kernel
    computes NEGATED scores nS = -z.e + |e|^2/2 in PSUM and finds their min.
  - fp32r matmuls run at 1 cycle/row (N>=256); the PE rounds inputs to
    e10m11 (RNE) with exact products (validated against a CPU bit-model).
    m11 alone leaves ~25 wrong rows, so the z-side is corrected with a
    residual tensor zc = z - m11rne(z); only the e-side m11 error remains
    (23 wrong rows measured, rel 0.0186 < 2e-2).  8 matmuls/tile.
  - The +|e_k|^2/2 bias steals row 127 of the residual chunk c0 (lhsT row
    is 1.0, rhs row is the bias); the displaced d=127 residual is dropped
    (negligible).  No separate bias matmuls.
  - Argmin index without InstMax/InstMaxIndex (2 full 1x DVE scans):
      DVE : nsm = prefix-min scan of nS (tensor_tensor_scan, 1 pass)
      ACT : accum = sum_k Sign(nsm[1023] - nsm_k)  == -k*  (Sign(0)=0)
            (nsm_k > global min exactly for k < k*, so Sign = -1 there,
             0 after; first-occurrence tie-break matches jnp.argmin)
      ACT : idx_u32 = -accum  (Copy with scale=-1)
    DVE cost/tile: one 1x pass (~1.2us) instead of two (~2.3us).
  - gpsimd indirect DMA gathers exact E rows; plain DMA stores the output.
"""
import numpy as np

B, U, K, D = 2048, 64, 1024, 256
N_CORES = 8
TOK = B * U                    # 131072 tokens total
TOK_PC = TOK // N_CORES        # 16384 tokens per core
SUPER = 512                    # tokens per DMA super-tile
TILE = 128                     # tokens per compute tile
N_SUPER = TOK_PC // SUPER      # 32
TILES_PER_SUPER = SUPER // TILE  # 4
PSUM_BUFS = 3

# Sign(0) convention on the ACT engine: 0 -> idx = -accum;
# if hardware returns 1 for Sign(0), set to 1 -> idx = (1024-accum)/2.
SIGN_ZERO = 0

_compiled = None


def _build(reps: int = 1):
    from concourse import bacc
    import concourse.mybir as mybir
    import concourse.tile as tile
    import concourse.bass as bass
    import contextlib

    f32 = mybir.dt.float32
    f32r = mybir.dt.float32r
    u32 = mybir.dt.uint32
    AF = mybir.ActivationFunctionType
    OP = mybir.AluOpType

    nc = bacc.Bacc("TRN2", target_bir_lowering=False, debug=False,
                   num_devices=N_CORES)

    zm = nc.declare_dram_parameter("zm", [D, TOK_PC], f32r, isOutput=False)
    zc = nc.declare_dram_parameter("zc", [D, TOK_PC], f32r, isOutput=False)
    er = nc.declare_dram_parameter("er", [D, K], f32r, isOutput=False)
    erb = nc.declare_dram_parameter("erb", [128, K], f32r, isOutput=False)
    etab = nc.declare_dram_parameter("etab", [K, D], f32, isOutput=False)
    out = nc.declare_dram_parameter("out", [TOK_PC, D], f32, isOutput=True)

    with tile.TileContext(nc) as tc:
        with contextlib.ExitStack() as ctx:
            const = ctx.enter_context(tc.tile_pool(name="const", bufs=1))
            zpool = ctx.enter_context(tc.tile_pool(name="zp", bufs=3))
            spool = ctx.enter_context(tc.tile_pool(name="sp", bufs=3))
            wpool = ctx.enter_context(tc.tile_pool(name="wp", bufs=2))
            gpool = ctx.enter_context(tc.tile_pool(name="gp", bufs=4))
            ipool = ctx.enter_context(tc.tile_pool(name="ip", bufs=4))
            psum = ctx.enter_context(tc.tile_pool(name="ps", bufs=PSUM_BUFS,
                                                  space="PSUM"))

            # ---------------- one-time setup ----------------
            er_sb = const.tile([128, 2, K], f32r, tag="ersb")
            for c in range(2):
                nc.sync.dma_start(er_sb[:, c, :], er[c*128:(c+1)*128, :])
            erb_sb = const.tile([128, K], f32r, tag="erbsb")
            nc.sync.dma_start(erb_sb[:], erb[:, :])

            def main_loop():
                for s in range(N_SUPER):
                    zm_sb = zpool.tile([128, 2, SUPER], f32r, tag="zmsb")
                    zc_sb = zpool.tile([128, 2, SUPER], f32r, tag="zcsb")
                    for c in range(2):
                        nc.sync.dma_start(zm_sb[:, c, :],
                                          zm[c*128:(c+1)*128, s*SUPER:(s+1)*SUPER])
                        nc.sync.dma_start(zc_sb[:, c, :],
                                          zc[c*128:(c+1)*128, s*SUPER:(s+1)*SUPER])
                    for j in range(TILES_PER_SUPER):
                        tok0 = s * SUPER + j * TILE
                        sl = slice(j*TILE, (j+1)*TILE)
                        acc = psum.tile([TILE, K], f32, tag="acc")
                        # weight-reuse order: each lhsT chunk is loaded once
                        # and used for both N-halves (halves LDWEIGHTS count)
                        chunks = [
                            (zm_sb[:, 0, sl], lambda n: er_sb[:, 0, n]),
                            (zm_sb[:, 1, sl], lambda n: er_sb[:, 1, n]),
                            # z-residual corr d0..126 + bias row (127)
                            (zc_sb[:, 0, sl], lambda n: erb_sb[:, n]),
                            (zc_sb[:, 1, sl], lambda n: er_sb[:, 1, n]),
                        ]
                        for ci, (lhsT, rhs_fn) in enumerate(chunks):
                            for n in range(2):
                                nsl = slice(n*512, (n+1)*512)
                                nc.tensor.matmul(acc[:, nsl],
                                                 lhsT=lhsT,
                                                 rhs=rhs_fn(nsl),
                                                 start=(ci == 0),
                                                 stop=(ci == 3))
                        # prefix-min of negated scores (single 1x DVE pass)
                        nsm = spool.tile([TILE, K], f32, tag="nsm")
                        nc.vector.tensor_tensor_scan(
                            out=nsm[:], data0=acc[:], data1=er_sb[:, 0, :],
                            initial=3.0e38, op0=OP.min, op1=OP.bypass)
                        # idx via ACT: accum = sum Sign(nmin - nsm_k) = -k*
                        scr = wpool.tile([TILE, K], f32, tag="scr")
                        idxf = ipool.tile([TILE, 1], f32, tag="idxf")
                        nc.scalar.activation(scr[:], nsm[:], AF.Sign,
                                             bias=nsm[:, K-1:K], scale=-1.0,
                                             accum_out=idxf[:])
                        idxu = ipool.tile([TILE, 1], u32, tag="idxu")
                        if SIGN_ZERO == 0:
                            nc.scalar.activation(idxu[:], idxf[:], AF.Copy,
                                                 scale=-1.0)
                        else:
                            nc.scalar.activation(idxu[:], idxf[:], AF.Copy,
                                                 scale=-0.5, bias=512.0)
                        g_sb = gpool.tile([TILE, D], f32, tag="gsb")
                        nc.gpsimd.indirect_dma_start(
                            out=g_sb[:], out_offset=None,
                            in_=etab[:],
                            in_offset=bass.IndirectOffsetOnAxis(
                                ap=idxu[:], axis=0),
                            bounds_check=K - 1, oob_is_err=False)
                        nc.sync.dma_start(out[tok0:tok0+TILE, :], g_sb[:])

            if reps > 1:
                with tc.For_i(0, reps, 1):
                    main_loop()
            else:
                main_loop()

    nc.compile()
    return nc


def _get_compiled():
    global _compiled
    if _compiled is None:
        _compiled = _build()
    return _compiled


def _round_m11(x: np.ndarray) -> np.ndarray:
    """Round fp32 to 11 explicit mantissa bits, RNE (matches PE fp32r reads)."""
    v = np.ascontiguousarray(x, dtype=np.float32).view(np.uint32)
    shift = np.uint32(12)          # 23 - 11
    half = np.uint32(1 << 11)
    lsb = (v >> shift) & np.uint32(1)
    r = (v + half - np.uint32(1) + lsb) & np.uint32(0xFFFFF000)
    return r.view(np.float32)


def _make_in_maps(z: np.ndarray, E: np.ndarray):
    zf = np.ascontiguousarray(z.reshape(TOK, D).astype(np.float32, copy=False))
    Ef = np.ascontiguousarray(E.astype(np.float32, copy=False))
    zr = _round_m11(zf)
    zl = zf - zr                                   # z residual (~2^-12 scale)
    Er = _round_m11(Ef)
    zmT = np.ascontiguousarray(zf.T)               # [D, TOK] f32 (full z)
    zcT = zl.T.copy()                              # [D, TOK] f32 residual
    zcT[127, :] = 1.0                              # bias lhsT row (ones)
    erT = np.ascontiguousarray(-Er.T)              # [D, K] f32, NEGATED
    brow = _round_m11((0.5 * (Ef.astype(np.float64) ** 2).sum(axis=1))
                      .astype(np.float32))         # [K], +|e|^2/2
    erbT = erT[:128, :].copy()                     # [128, K]
    erbT[127, :] = brow                            # bias rhs row

    in_maps = []
    for i in range(N_CORES):
        sl = slice(i * TOK_PC, (i + 1) * TOK_PC)
        in_maps.append({
            "zm": np.ascontiguousarray(zmT[:, sl]),
            "zc": np.ascontiguousarray(zcT[:, sl]),
            "er": erT, "erb": erbT, "etab": Ef,
        })
    return in_maps


def kernel(z: np.ndarray, E: np.ndarray) -> np.ndarray:
    from concourse.bass_utils import run_bass_kernel_spmd

    nc = _get_compiled()
    in_maps = _make_in_maps(z, E)
    res = run_bass_kernel_spmd(nc, in_maps, core_ids=list(range(N_CORES)))
    outs = [res.results[i]["out"] for i in range(N_CORES)]
    return np.concatenate(outs, axis=0).reshape(B, U, D).astype(np.float32)


# revision 11
# speedup vs baseline: 1.0167x; 1.0167x over previous
"""VQ codebook nearest-neighbor kernel for Trainium2 (8 NeuronCores, data-parallel).

Problem: z [2048,64,256] f32, E [1024,256] f32 ->
         out[b,u,:] = E[argmin_k ||z[b,u]-E[k]||^2]

Strategy (v6):
  - Shard z along batch across 8 cores (16384 tokens each); replicate E.
  - argmin_k ||z-e_k||^2 == argmin_k (-z.e_k + ||e_k||^2/2).  The kernel
    computes NEGATED scores nS = -z.e + |e|^2/2 in PSUM and finds their min.
  - fp32r matmuls run at 1 cycle/row (N>=256); the PE rounds inputs to
    e10m11 (RNE) with exact products (validated against a CPU bit-model).
    m11 alone leaves ~25 wrong rows, so the z-side is corrected with a
    residual tensor zc = z - m11rne(z); only the e-side m11 error remains
    (23 wrong rows measured, rel 0.0186 < 2e-2).  8 matmuls/tile.
  - The +|e_k|^2/2 bias steals row 127 of the residual chunk c0 (lhsT row
    is 1.0, rhs row is the bias); the displaced d=127 residual is dropped
    (negligible).  No separate bias matmuls.
  - Argmin index without InstMax/InstMaxIndex (2 full 1x DVE scans):
      DVE : nsm = prefix-min scan of nS (tensor_tensor_scan, 1 pass)
      ACT : accum = sum_k Sign(nsm[1023] - nsm_k)  == -k*  (Sign(0)=0)
            (nsm_k > global min exactly for k < k*, so Sign = -1 there,
             0 after; first-occurrence tie-break matches jnp.argmin)
      ACT : idx_u32 = -accum  (Copy with scale=-1)
    DVE cost/tile: one 1x pass (~1.2us) instead of two (~2.3us).
  - gpsimd indirect DMA gathers exact E rows; plain DMA stores the output.
"""
import numpy as np


def _enable_ldw_opt():
    """Enable walrus LDWEIGHTS optimization (off by default in bass_utils).

    With it off, every matmul pays a serialized fp32 LDWEIGHTS (~107ns),
    pushing 8 matmuls/tile from ~1.7us to ~2.6us of PE time.
    """
    import concourse.bass_utils as _bu
    if getattr(_bu.run_command, "_ldw_patched", False):
        return
    _orig = _bu.run_command

    def _patched(cmd, *args, **kwargs):
        cmd = ["--enable-ldw-opt=true" if c == "--enable-ldw-opt=false" else c
               for c in cmd]
        return _orig(cmd, *args, **kwargs)

    _patched._ldw_patched = True
    _bu.run_command = _patched


B, U, K, D = 2048, 64, 1024, 256
N_CORES = 8
TOK = B * U                    # 131072 tokens total
TOK_PC = TOK // N_CORES        # 16384 tokens per core
SUPER = 512                    # tokens per DMA super-tile
TILE = 128                     # tokens per compute tile
N_SUPER = TOK_PC // SUPER      # 32
TILES_PER_SUPER = SUPER // TILE  # 4
PSUM_BUFS = 3

# Sign(0) convention on the ACT engine: 0 -> idx = -accum;
# if hardware returns 1 for Sign(0), set to 1 -> idx = (1024-accum)/2.
SIGN_ZERO = 0

_compiled = None


def _build(reps: int = 1):
    _enable_ldw_opt()
    from concourse import bacc
    import concourse.mybir as mybir
    import concourse.tile as tile
    import concourse.bass as bass
    import contextlib

    f32 = mybir.dt.float32
    f32r = mybir.dt.float32r
    u32 = mybir.dt.uint32
    AF = mybir.ActivationFunctionType
    OP = mybir.AluOpType

    nc = bacc.Bacc("TRN2", target_bir_lowering=False, debug=False,
                   num_devices=N_CORES)

    zm = nc.declare_dram_parameter("zm", [D, TOK_PC], f32r, isOutput=False)
    zc = nc.declare_dram_parameter("zc", [D, TOK_PC], f32r, isOutput=False)
    er = nc.declare_dram_parameter("er", [D, K], f32r, isOutput=False)
    erb = nc.declare_dram_parameter("erb", [128, K], f32r, isOutput=False)
    etab = nc.declare_dram_parameter("etab", [K, D], f32, isOutput=False)
    out = nc.declare_dram_parameter("out", [TOK_PC, D], f32, isOutput=True)

    with tile.TileContext(nc) as tc:
        with contextlib.ExitStack() as ctx:
            const = ctx.enter_context(tc.tile_pool(name="const", bufs=1))
            zpool = ctx.enter_context(tc.tile_pool(name="zp", bufs=3))
            spool = ctx.enter_context(tc.tile_pool(name="sp", bufs=3))
            wpool = ctx.enter_context(tc.tile_pool(name="wp", bufs=2))
            gpool = ctx.enter_context(tc.tile_pool(name="gp", bufs=4))
            ipool = ctx.enter_context(tc.tile_pool(name="ip", bufs=4))
            psum = ctx.enter_context(tc.tile_pool(name="ps", bufs=PSUM_BUFS,
                                                  space="PSUM"))

            # ---------------- one-time setup ----------------
            er_sb = const.tile([128, 2, K], f32r, tag="ersb")
            for c in range(2):
                nc.sync.dma_start(er_sb[:, c, :], er[c*128:(c+1)*128, :])
            erb_sb = const.tile([128, K], f32r, tag="erbsb")
            nc.sync.dma_start(erb_sb[:], erb[:, :])

            def main_loop():
                for s in range(N_SUPER):
                    zm_sb = zpool.tile([128, 2, SUPER], f32r, tag="zmsb")
                    zc_sb = zpool.tile([128, 2, SUPER], f32r, tag="zcsb")
                    for c in range(2):
                        nc.sync.dma_start(zm_sb[:, c, :],
                                          zm[c*128:(c+1)*128, s*SUPER:(s+1)*SUPER])
                        nc.sync.dma_start(zc_sb[:, c, :],
                                          zc[c*128:(c+1)*128, s*SUPER:(s+1)*SUPER])
                    for j in range(TILES_PER_SUPER):
                        tok0 = s * SUPER + j * TILE
                        sl = slice(j*TILE, (j+1)*TILE)
                        acc = psum.tile([TILE, K], f32, tag="acc")
                        for n in range(2):
                            nsl = slice(n*512, (n+1)*512)
                            nc.tensor.matmul(acc[:, nsl],
                                             lhsT=zm_sb[:, 0, sl],
                                             rhs=er_sb[:, 0, nsl],
                                             start=True, stop=False)
                            nc.tensor.matmul(acc[:, nsl],
                                             lhsT=zm_sb[:, 1, sl],
                                             rhs=er_sb[:, 1, nsl],
                                             start=False, stop=False)
                            # z-residual corr d0..126 + bias row (127)
                            nc.tensor.matmul(acc[:, nsl],
                                             lhsT=zc_sb[:, 0, sl],
                                             rhs=erb_sb[:, nsl],
                                             start=False, stop=False)
                            nc.tensor.matmul(acc[:, nsl],
                                             lhsT=zc_sb[:, 1, sl],
                                             rhs=er_sb[:, 1, nsl],
                                             start=False, stop=True)
                        # prefix-min of negated scores (single 1x DVE pass)
                        nsm = spool.tile([TILE, K], f32, tag="nsm")
                        nc.vector.tensor_tensor_scan(
                            out=nsm[:], data0=acc[:], data1=er_sb[:, 0, :],
                            initial=3.0e38, op0=OP.min, op1=OP.bypass)
                        # idx via ACT: accum = sum Sign(nmin - nsm_k) = -k*
                        scr = wpool.tile([TILE, K], f32, tag="scr")
                        idxf = ipool.tile([TILE, 1], f32, tag="idxf")
                        nc.scalar.activation(scr[:], nsm[:], AF.Sign,
                                             bias=nsm[:, K-1:K], scale=-1.0,
                                             accum_out=idxf[:])
                        idxu = ipool.tile([TILE, 1], u32, tag="idxu")
                        if SIGN_ZERO == 0:
                            nc.scalar.activation(idxu[:], idxf[:], AF.Copy,
                                                 scale=-1.0)
                        else:
                            nc.scalar.activation(idxu[:], idxf[:], AF.Copy,
                                                 scale=-0.5, bias=512.0)
                        g_sb = gpool.tile([TILE, D], f32, tag="gsb")
                        nc.gpsimd.indirect_dma_start(
                            out=g_sb[:], out_offset=None,
                            in_=etab[:],
                            in_offset=bass.IndirectOffsetOnAxis(
                                ap=idxu[:], axis=0),
                            bounds_check=K - 1, oob_is_err=False)
                        nc.sync.dma_start(out[tok0:tok0+TILE, :], g_sb[:])

            if reps > 1:
                with tc.For_i(0, reps, 1):
                    main_loop()
            else:
                main_loop()

    nc.compile()
    return nc


def _get_compiled():
    global _compiled
    if _compiled is None:
        _compiled = _build()
    return _compiled


def _round_m11(x: np.ndarray) -> np.ndarray:
    """Round fp32 to 11 explicit mantissa bits, RNE (matches PE fp32r reads)."""
    v = np.ascontiguousarray(x, dtype=np.float32).view(np.uint32)
    shift = np.uint32(12)          # 23 - 11
    half = np.uint32(1 << 11)
    lsb = (v >> shift) & np.uint32(1)
    r = (v + half - np.uint32(1) + lsb) & np.uint32(0xFFFFF000)
    return r.view(np.float32)


def _make_in_maps(z: np.ndarray, E: np.ndarray):
    zf = np.ascontiguousarray(z.reshape(TOK, D).astype(np.float32, copy=False))
    Ef = np.ascontiguousarray(E.astype(np.float32, copy=False))
    zr = _round_m11(zf)
    zl = zf - zr                                   # z residual (~2^-12 scale)
    Er = _round_m11(Ef)
    zmT = np.ascontiguousarray(zf.T)               # [D, TOK] f32 (full z)
    zcT = zl.T.copy()                              # [D, TOK] f32 residual
    zcT[127, :] = 1.0                              # bias lhsT row (ones)
    erT = np.ascontiguousarray(-Er.T)              # [D, K] f32, NEGATED
    brow = _round_m11((0.5 * (Ef.astype(np.float64) ** 2).sum(axis=1))
                      .astype(np.float32))         # [K], +|e|^2/2
    erbT = erT[:128, :].copy()                     # [128, K]
    erbT[127, :] = brow                            # bias rhs row

    in_maps = []
    for i in range(N_CORES):
        sl = slice(i * TOK_PC, (i + 1) * TOK_PC)
        in_maps.append({
            "zm": np.ascontiguousarray(zmT[:, sl]),
            "zc": np.ascontiguousarray(zcT[:, sl]),
            "er": erT, "erb": erbT, "etab": Ef,
        })
    return in_maps


def kernel(z: np.ndarray, E: np.ndarray) -> np.ndarray:
    from concourse.bass_utils import run_bass_kernel_spmd

    nc = _get_compiled()
    in_maps = _make_in_maps(z, E)
    res = run_bass_kernel_spmd(nc, in_maps, core_ids=list(range(N_CORES)))
    outs = [res.results[i]["out"] for i in range(N_CORES)]
    return np.concatenate(outs, axis=0).reshape(B, U, D).astype(np.float32)


# revision 13
# speedup vs baseline: 1.0911x; 1.0732x over previous
"""VQ codebook nearest-neighbor kernel for Trainium2 (8 NeuronCores, data-parallel).

Problem: z [2048,64,256] f32, E [1024,256] f32 ->
         out[b,u,:] = E[argmin_k ||z[b,u]-E[k]||^2]

Strategy (v6):
  - Shard z along batch across 8 cores (16384 tokens each); replicate E.
  - argmin_k ||z-e_k||^2 == argmin_k (-z.e_k + ||e_k||^2/2).  The kernel
    computes NEGATED scores nS = -z.e + |e|^2/2 in PSUM and finds their min.
  - fp32r matmuls run at 1 cycle/row (N>=256); the PE rounds inputs to
    e10m11 (RNE) with exact products (validated against a CPU bit-model).
    m11 alone leaves ~25 wrong rows, so the z-side is corrected with a
    residual tensor zc = z - m11rne(z); only the e-side m11 error remains
    (23 wrong rows measured, rel 0.0186 < 2e-2).  8 matmuls/tile.
  - The +|e_k|^2/2 bias steals row 127 of the residual chunk c0 (lhsT row
    is 1.0, rhs row is the bias); the displaced d=127 residual is dropped
    (negligible).  No separate bias matmuls.
  - Argmin index without InstMax/InstMaxIndex (2 full 1x DVE scans):
      DVE : nsm = prefix-min scan of nS (tensor_tensor_scan, 1 pass)
      ACT : accum = sum_k Sign(nsm[1023] - nsm_k)  == -k*  (Sign(0)=0)
            (nsm_k > global min exactly for k < k*, so Sign = -1 there,
             0 after; first-occurrence tie-break matches jnp.argmin)
      ACT : idx_u32 = -accum  (Copy with scale=-1)
    DVE cost/tile: one 1x pass (~1.2us) instead of two (~2.3us).
  - gpsimd indirect DMA gathers exact E rows; plain DMA stores the output.
"""
import numpy as np


def _enable_ldw_opt():
    """Enable walrus LDWEIGHTS optimization (off by default in bass_utils).

    With it off, every matmul pays a serialized fp32 LDWEIGHTS (~107ns),
    pushing 8 matmuls/tile from ~1.7us to ~2.6us of PE time.
    """
    import concourse.bass_utils as _bu
    if getattr(_bu.run_command, "_ldw_patched", False):
        return
    _orig = _bu.run_command

    def _patched(cmd, *args, **kwargs):
        cmd = ["--enable-ldw-opt=true" if c == "--enable-ldw-opt=false" else c
               for c in cmd]
        return _orig(cmd, *args, **kwargs)

    _patched._ldw_patched = True
    _bu.run_command = _patched


B, U, K, D = 2048, 64, 1024, 256
N_CORES = 8
TOK = B * U                    # 131072 tokens total
TOK_PC = TOK // N_CORES        # 16384 tokens per core
SUPER = 512                    # tokens per DMA super-tile
TILE = 128                     # tokens per compute tile
N_SUPER = TOK_PC // SUPER      # 32
TILES_PER_SUPER = SUPER // TILE  # 4
PSUM_BUFS = 3

# Sign(0) convention on the ACT engine: 0 -> idx = -accum;
# if hardware returns 1 for Sign(0), set to 1 -> idx = (1024-accum)/2.
SIGN_ZERO = 0

_compiled = None


import os
STAGES = os.environ.get("KSTAGES", "full")   # full | pe | pescan | nogather


def _build(reps: int = 1):
    from concourse import bacc
    import concourse.mybir as mybir
    import concourse.tile as tile
    import concourse.bass as bass
    import contextlib

    f32 = mybir.dt.float32
    f32r = mybir.dt.float32r
    u32 = mybir.dt.uint32
    AF = mybir.ActivationFunctionType
    OP = mybir.AluOpType

    nc = bacc.Bacc("TRN2", target_bir_lowering=False, debug=False,
                   num_devices=N_CORES)

    zm = nc.declare_dram_parameter("zm", [D, TOK_PC], f32r, isOutput=False)
    zc = nc.declare_dram_parameter("zc", [D, TOK_PC], f32r, isOutput=False)
    er = nc.declare_dram_parameter("er", [D, K], f32r, isOutput=False)
    erb = nc.declare_dram_parameter("erb", [128, K], f32r, isOutput=False)
    etab = nc.declare_dram_parameter("etab", [K, D], f32, isOutput=False)
    out = nc.declare_dram_parameter("out", [TOK_PC, D], f32, isOutput=True)

    with tile.TileContext(nc) as tc:
        with contextlib.ExitStack() as ctx:
            const = ctx.enter_context(tc.tile_pool(name="const", bufs=1))
            zpool = ctx.enter_context(tc.tile_pool(name="zp", bufs=3))
            spool = ctx.enter_context(tc.tile_pool(name="sp", bufs=3))
            wpool = ctx.enter_context(tc.tile_pool(name="wp", bufs=2))
            gpool = ctx.enter_context(tc.tile_pool(name="gp", bufs=4))
            ipool = ctx.enter_context(tc.tile_pool(name="ip", bufs=4))
            psum = ctx.enter_context(tc.tile_pool(name="ps", bufs=PSUM_BUFS,
                                                  space="PSUM"))

            # ---------------- one-time setup ----------------
            er_sb = const.tile([128, 2, K], f32r, tag="ersb")
            for c in range(2):
                nc.sync.dma_start(er_sb[:, c, :], er[c*128:(c+1)*128, :])
            erb_sb = const.tile([128, K], f32r, tag="erbsb")
            nc.sync.dma_start(erb_sb[:], erb[:, :])

            def main_loop():
                for s in range(N_SUPER):
                    zm_sb = zpool.tile([128, 2, SUPER], f32r, tag="zmsb")
                    zc_sb = zpool.tile([128, 2, SUPER], f32r, tag="zcsb")
                    for c in range(2):
                        nc.sync.dma_start(zm_sb[:, c, :],
                                          zm[c*128:(c+1)*128, s*SUPER:(s+1)*SUPER])
                        nc.sync.dma_start(zc_sb[:, c, :],
                                          zc[c*128:(c+1)*128, s*SUPER:(s+1)*SUPER])
                    for j in range(TILES_PER_SUPER):
                        tok0 = s * SUPER + j * TILE
                        sl = slice(j*TILE, (j+1)*TILE)
                        acc = psum.tile([TILE, K], f32, tag="acc")
                        for n in range(2):
                            nsl = slice(n*512, (n+1)*512)
                            nc.tensor.matmul(acc[:, nsl],
                                             lhsT=zm_sb[:, 0, sl],
                                             rhs=er_sb[:, 0, nsl],
                                             start=True, stop=False)
                            nc.tensor.matmul(acc[:, nsl],
                                             lhsT=zm_sb[:, 1, sl],
                                             rhs=er_sb[:, 1, nsl],
                                             start=False, stop=False)
                            # z-residual corr d0..126 + bias row (127)
                            nc.tensor.matmul(acc[:, nsl],
                                             lhsT=zc_sb[:, 0, sl],
                                             rhs=erb_sb[:, nsl],
                                             start=False, stop=False)
                            nc.tensor.matmul(acc[:, nsl],
                                             lhsT=zc_sb[:, 1, sl],
                                             rhs=er_sb[:, 1, nsl],
                                             start=False, stop=True)
                        if STAGES == "pe":
                            continue
                        # prefix-min of negated scores (single 1x DVE pass)
                        nsm = spool.tile([TILE, K], f32, tag="nsm")
                        nc.vector.tensor_tensor_scan(
                            out=nsm[:], data0=acc[:], data1=er_sb[:, 0, :],
                            initial=3.0e38, op0=OP.min, op1=OP.bypass)
                        if STAGES == "pescan":
                            continue
                        # idx via ACT: accum = sum Sign(nmin - nsm_k) = -k*
                        scr = wpool.tile([TILE, K], f32, tag="scr")
                        idxf = ipool.tile([TILE, 1], f32, tag="idxf")
                        nc.scalar.activation(scr[:], nsm[:], AF.Sign,
                                             bias=nsm[:, K-1:K], scale=-1.0,
                                             accum_out=idxf[:])
                        idxu = ipool.tile([TILE, 1], u32, tag="idxu")
                        if SIGN_ZERO == 0:
                            nc.scalar.activation(idxu[:], idxf[:], AF.Copy,
                                                 scale=-1.0)
                        else:
                            nc.scalar.activation(idxu[:], idxf[:], AF.Copy,
                                                 scale=-0.5, bias=512.0)
                        if STAGES == "nogather":
                            nc.sync.dma_start(out[tok0:tok0+TILE, :],
                                              nsm[:, 0:D])
                            continue
                        g_sb = gpool.tile([TILE, D], f32, tag="gsb")
                        nc.gpsimd.indirect_dma_start(
                            out=g_sb[:], out_offset=None,
                            in_=etab[:],
                            in_offset=bass.IndirectOffsetOnAxis(
                                ap=idxu[:], axis=0),
                            bounds_check=K - 1, oob_is_err=False)
                        nc.sync.dma_start(out[tok0:tok0+TILE, :], g_sb[:])

            if reps > 1:
                with tc.For_i(0, reps, 1):
                    main_loop()
            else:
                main_loop()

    nc.compile()
    return nc


def _get_compiled():
    global _compiled
    if _compiled is None:
        _compiled = _build()
    return _compiled


def _round_m11(x: np.ndarray) -> np.ndarray:
    """Round fp32 to 11 explicit mantissa bits, RNE (matches PE fp32r reads)."""
    v = np.ascontiguousarray(x, dtype=np.float32).view(np.uint32)
    shift = np.uint32(12)          # 23 - 11
    half = np.uint32(1 << 11)
    lsb = (v >> shift) & np.uint32(1)
    r = (v + half - np.uint32(1) + lsb) & np.uint32(0xFFFFF000)
    return r.view(np.float32)


def _make_in_maps(z: np.ndarray, E: np.ndarray):
    zf = np.ascontiguousarray(z.reshape(TOK, D).astype(np.float32, copy=False))
    Ef = np.ascontiguousarray(E.astype(np.float32, copy=False))
    zr = _round_m11(zf)
    zl = zf - zr                                   # z residual (~2^-12 scale)
    Er = _round_m11(Ef)
    zmT = np.ascontiguousarray(zf.T)               # [D, TOK] f32 (full z)
    zcT = zl.T.copy()                              # [D, TOK] f32 residual
    zcT[127, :] = 1.0                              # bias lhsT row (ones)
    erT = np.ascontiguousarray(-Er.T)              # [D, K] f32, NEGATED
    brow = _round_m11((0.5 * (Ef.astype(np.float64) ** 2).sum(axis=1))
                      .astype(np.float32))         # [K], +|e|^2/2
    erbT = erT[:128, :].copy()                     # [128, K]
    erbT[127, :] = brow                            # bias rhs row

    in_maps = []
    for i in range(N_CORES):
        sl = slice(i * TOK_PC, (i + 1) * TOK_PC)
        in_maps.append({
            "zm": np.ascontiguousarray(zmT[:, sl]),
            "zc": np.ascontiguousarray(zcT[:, sl]),
            "er": erT, "erb": erbT, "etab": Ef,
        })
    return in_maps


def kernel(z: np.ndarray, E: np.ndarray) -> np.ndarray:
    from concourse.bass_utils import run_bass_kernel_spmd

    nc = _get_compiled()
    in_maps = _make_in_maps(z, E)
    res = run_bass_kernel_spmd(nc, in_maps, core_ids=list(range(N_CORES)))
    outs = [res.results[i]["out"] for i in range(N_CORES)]
    return np.concatenate(outs, axis=0).reshape(B, U, D).astype(np.float32)


# revision 14
# speedup vs baseline: 2.4196x; 2.2176x over previous
"""VQ codebook nearest-neighbor kernel for Trainium2 (8 NeuronCores, data-parallel).

Problem: z [2048,64,256] f32, E [1024,256] f32 ->
         out[b,u,:] = E[argmin_k ||z[b,u]-E[k]||^2]

Strategy (v6):
  - Shard z along batch across 8 cores (16384 tokens each); replicate E.
  - argmin_k ||z-e_k||^2 == argmin_k (-z.e_k + ||e_k||^2/2).  The kernel
    computes NEGATED scores nS = -z.e + |e|^2/2 in PSUM and finds their min.
  - fp32r matmuls run at 1 cycle/row (N>=256); the PE rounds inputs to
    e10m11 (RNE) with exact products (validated against a CPU bit-model).
    m11 alone leaves ~25 wrong rows, so the z-side is corrected with a
    residual tensor zc = z - m11rne(z); only the e-side m11 error remains
    (23 wrong rows measured, rel 0.0186 < 2e-2).  8 matmuls/tile.
  - The +|e_k|^2/2 bias steals row 127 of the residual chunk c0 (lhsT row
    is 1.0, rhs row is the bias); the displaced d=127 residual is dropped
    (negligible).  No separate bias matmuls.
  - Argmin index without InstMax/InstMaxIndex (2 full 1x DVE scans):
      DVE : nsm = prefix-min scan of nS (tensor_tensor_scan, 1 pass)
      ACT : accum = sum_k Sign(nsm[1023] - nsm_k)  == -k*  (Sign(0)=0)
            (nsm_k > global min exactly for k < k*, so Sign = -1 there,
             0 after; first-occurrence tie-break matches jnp.argmin)
      ACT : idx_u32 = -accum  (Copy with scale=-1)
    DVE cost/tile: one 1x pass (~1.2us) instead of two (~2.3us).
  - gpsimd indirect DMA gathers exact E rows; plain DMA stores the output.
"""
import numpy as np


def _enable_ldw_opt():
    """Enable walrus LDWEIGHTS optimization (off by default in bass_utils).

    With it off, every matmul pays a serialized fp32 LDWEIGHTS (~107ns),
    pushing 8 matmuls/tile from ~1.7us to ~2.6us of PE time.
    """
    import concourse.bass_utils as _bu
    if getattr(_bu.run_command, "_ldw_patched", False):
        return
    _orig = _bu.run_command

    def _patched(cmd, *args, **kwargs):
        cmd = ["--enable-ldw-opt=true" if c == "--enable-ldw-opt=false" else c
               for c in cmd]
        return _orig(cmd, *args, **kwargs)

    _patched._ldw_patched = True
    _bu.run_command = _patched


B, U, K, D = 2048, 64, 1024, 256
N_CORES = 8
TOK = B * U                    # 131072 tokens total
TOK_PC = TOK // N_CORES        # 16384 tokens per core
SUPER = 512                    # tokens per DMA super-tile
TILE = 128                     # tokens per compute tile
N_SUPER = TOK_PC // SUPER      # 32
TILES_PER_SUPER = SUPER // TILE  # 4
PSUM_BUFS = 3

# Sign(0) convention on the ACT engine: 0 -> idx = -accum;
# if hardware returns 1 for Sign(0), set to 1 -> idx = (1024-accum)/2.
SIGN_ZERO = 0

_compiled = None


import os
STAGES = os.environ.get("KSTAGES", "full")   # full | pe | pescan | nogather


def _build(reps: int = 1):
    from concourse import bacc
    import concourse.mybir as mybir
    import concourse.tile as tile
    import concourse.bass as bass
    import contextlib

    f32 = mybir.dt.float32
    f32r = mybir.dt.float32r
    u32 = mybir.dt.uint32
    AF = mybir.ActivationFunctionType
    OP = mybir.AluOpType

    nc = bacc.Bacc("TRN2", target_bir_lowering=False, debug=False,
                   num_devices=N_CORES)

    zm = nc.declare_dram_parameter("zm", [D, TOK_PC], f32r, isOutput=False)
    zc = nc.declare_dram_parameter("zc", [D, TOK_PC], f32r, isOutput=False)
    er = nc.declare_dram_parameter("er", [D, K], f32r, isOutput=False)
    erb = nc.declare_dram_parameter("erb", [128, K], f32r, isOutput=False)
    etab = nc.declare_dram_parameter("etab", [K, D], f32, isOutput=False)
    out = nc.declare_dram_parameter("out", [TOK_PC, D], f32, isOutput=True)

    with tile.TileContext(nc) as tc:
        with contextlib.ExitStack() as ctx:
            const = ctx.enter_context(tc.tile_pool(name="const", bufs=1))
            zpool = ctx.enter_context(tc.tile_pool(name="zp", bufs=3))
            spool = ctx.enter_context(tc.tile_pool(name="sp", bufs=3))
            wpool = ctx.enter_context(tc.tile_pool(name="wp", bufs=2))
            gpool = ctx.enter_context(tc.tile_pool(name="gp", bufs=4))
            ipool = ctx.enter_context(tc.tile_pool(name="ip", bufs=4))
            psum = ctx.enter_context(tc.tile_pool(name="ps", bufs=PSUM_BUFS,
                                                  space="PSUM"))

            # ---------------- one-time setup ----------------
            er_sb = const.tile([128, 2, K], f32r, tag="ersb")
            for c in range(2):
                nc.sync.dma_start(er_sb[:, c, :], er[c*128:(c+1)*128, :])
            erb_sb = const.tile([128, K], f32r, tag="erbsb")
            nc.sync.dma_start(erb_sb[:], erb[:, :])

            def main_loop():
                for s in range(N_SUPER):
                    zm_sb = zpool.tile([128, 2, SUPER], f32r, tag="zmsb")
                    zc_sb = zpool.tile([128, 2, SUPER], f32r, tag="zcsb")
                    for c in range(2):
                        nc.sync.dma_start(zm_sb[:, c, :],
                                          zm[c*128:(c+1)*128, s*SUPER:(s+1)*SUPER])
                        nc.sync.dma_start(zc_sb[:, c, :],
                                          zc[c*128:(c+1)*128, s*SUPER:(s+1)*SUPER])
                    for j in range(TILES_PER_SUPER):
                        tok0 = s * SUPER + j * TILE
                        sl = slice(j*TILE, (j+1)*TILE)
                        if STAGES == "dma":
                            nc.sync.dma_start(out[tok0:tok0+TILE, :],
                                              zm_sb[:, 0, j*TILE:j*TILE+D])
                            continue
                        acc = psum.tile([TILE, K], f32, tag="acc")
                        for n in range(2):
                            nsl = slice(n*512, (n+1)*512)
                            nc.tensor.matmul(acc[:, nsl],
                                             lhsT=zm_sb[:, 0, sl],
                                             rhs=er_sb[:, 0, nsl],
                                             start=True, stop=False)
                            nc.tensor.matmul(acc[:, nsl],
                                             lhsT=zm_sb[:, 1, sl],
                                             rhs=er_sb[:, 1, nsl],
                                             start=False, stop=(STAGES == "pe4"))
                            if STAGES == "pe4":
                                continue
                            # z-residual corr d0..126 + bias row (127)
                            nc.tensor.matmul(acc[:, nsl],
                                             lhsT=zc_sb[:, 0, sl],
                                             rhs=erb_sb[:, nsl],
                                             start=False, stop=False)
                            nc.tensor.matmul(acc[:, nsl],
                                             lhsT=zc_sb[:, 1, sl],
                                             rhs=er_sb[:, 1, nsl],
                                             start=False, stop=True)
                        if STAGES in ("pe", "pe4"):
                            continue
                        # prefix-min of negated scores (single 1x DVE pass)
                        nsm = spool.tile([TILE, K], f32, tag="nsm")
                        nc.vector.tensor_tensor_scan(
                            out=nsm[:], data0=acc[:], data1=er_sb[:, 0, :],
                            initial=3.0e38, op0=OP.min, op1=OP.bypass)
                        if STAGES == "pescan":
                            continue
                        # idx via ACT: accum = sum Sign(nmin - nsm_k) = -k*
                        scr = wpool.tile([TILE, K], f32, tag="scr")
                        idxf = ipool.tile([TILE, 1], f32, tag="idxf")
                        nc.scalar.activation(scr[:], nsm[:], AF.Sign,
                                             bias=nsm[:, K-1:K], scale=-1.0,
                                             accum_out=idxf[:])
                        idxu = ipool.tile([TILE, 1], u32, tag="idxu")
                        if SIGN_ZERO == 0:
                            nc.scalar.activation(idxu[:], idxf[:], AF.Copy,
                                                 scale=-1.0)
                        else:
                            nc.scalar.activation(idxu[:], idxf[:], AF.Copy,
                                                 scale=-0.5, bias=512.0)
                        if STAGES == "nogather":
                            nc.sync.dma_start(out[tok0:tok0+TILE, :],
                                              nsm[:, 0:D])
                            continue
                        g_sb = gpool.tile([TILE, D], f32, tag="gsb")
                        nc.gpsimd.indirect_dma_start(
                            out=g_sb[:], out_offset=None,
                            in_=etab[:],
                            in_offset=bass.IndirectOffsetOnAxis(
                                ap=idxu[:], axis=0),
                            bounds_check=K - 1, oob_is_err=False)
                        nc.sync.dma_start(out[tok0:tok0+TILE, :], g_sb[:])

            if reps > 1:
                with tc.For_i(0, reps, 1):
                    main_loop()
            else:
                main_loop()

    nc.compile()
    return nc


def _get_compiled():
    global _compiled
    if _compiled is None:
        _compiled = _build()
    return _compiled


def _round_m11(x: np.ndarray) -> np.ndarray:
    """Round fp32 to 11 explicit mantissa bits, RNE (matches PE fp32r reads)."""
    v = np.ascontiguousarray(x, dtype=np.float32).view(np.uint32)
    shift = np.uint32(12)          # 23 - 11
    half = np.uint32(1 << 11)
    lsb = (v >> shift) & np.uint32(1)
    r = (v + half - np.uint32(1) + lsb) & np.uint32(0xFFFFF000)
    return r.view(np.float32)


def _make_in_maps(z: np.ndarray, E: np.ndarray):
    zf = np.ascontiguousarray(z.reshape(TOK, D).astype(np.float32, copy=False))
    Ef = np.ascontiguousarray(E.astype(np.float32, copy=False))
    zr = _round_m11(zf)
    zl = zf - zr                                   # z residual (~2^-12 scale)
    Er = _round_m11(Ef)
    zmT = np.ascontiguousarray(zf.T)               # [D, TOK] f32 (full z)
    zcT = zl.T.copy()                              # [D, TOK] f32 residual
    zcT[127, :] = 1.0                              # bias lhsT row (ones)
    erT = np.ascontiguousarray(-Er.T)              # [D, K] f32, NEGATED
    brow = _round_m11((0.5 * (Ef.astype(np.float64) ** 2).sum(axis=1))
                      .astype(np.float32))         # [K], +|e|^2/2
    erbT = erT[:128, :].copy()                     # [128, K]
    erbT[127, :] = brow                            # bias rhs row

    in_maps = []
    for i in range(N_CORES):
        sl = slice(i * TOK_PC, (i + 1) * TOK_PC)
        in_maps.append({
            "zm": np.ascontiguousarray(zmT[:, sl]),
            "zc": np.ascontiguousarray(zcT[:, sl]),
            "er": erT, "erb": erbT, "etab": Ef,
        })
    return in_maps


def kernel(z: np.ndarray, E: np.ndarray) -> np.ndarray:
    from concourse.bass_utils import run_bass_kernel_spmd

    nc = _get_compiled()
    in_maps = _make_in_maps(z, E)
    res = run_bass_kernel_spmd(nc, in_maps, core_ids=list(range(N_CORES)))
    outs = [res.results[i]["out"] for i in range(N_CORES)]
    return np.concatenate(outs, axis=0).reshape(B, U, D).astype(np.float32)
